# revision 2
# baseline (speedup 1.0000x reference)
"""Corr3D via K-packed block-diagonal TensorEngine Grams (v4).

Same compute scheme as v3 (supers of 4 w-adjacent (4h,4w,2t) blocks, K=128
block-diagonal lhsT, one N=144 matmul per 128 voxels), with two fixes:

1. The block-diagonal weights are materialized on the host in DRAM
   ([128x128] bf16 per super, zeros included) and DMAed in 4-super batches
   (1KB runs, one DMA per quad) instead of 6720 tiny diagonal-block DMAs.
2. m-order is v*4+g (voxel-major), so PSUM partition halves need only a
   96-wide column band (lower half: cols 0..96, upper: 48..144) out of 144.
   Eviction writes just the bands: raw output drops 62 -> 41 MB/core.

Layout per core:
  columns: (hb<5, ws<6); quads tq<14 of supers s<4 (tb = 4*tq+s, t0 = 2*tb)
  m = v*4 + g, v = i*8 + j*2 + l  (i<4 h, j<4 w, l<2 t)
  gram col n = (i+dh)*24 + (j+dw)*4 + (l+dt); band offset 48 for v>=16
"""

import sys

if "/opt/trn_rl_repo" not in sys.path:
    sys.path.insert(0, "/opt/trn_rl_repo")

import numpy as np
import ml_dtypes

B, C, H, W, T = 2, 32, 80, 96, 112
N_CORES = 8
HR = 20                    # h rows per core
HB, WS, TB = 5, 6, 56      # h-supers, w-supers, t-blocks per core
AH, AW, AT = 4, 4, 2       # block shape (32 voxels)
SH, SW, ST = 6, 6, 114     # quadrant slab shape (padded t)
NW = 6 * 6 * 4             # 144 window cols per super
BW = 96                    # evicted band width per partition half
KF = SH * SW * ST          # 4104 slab elems per slab-row
EV = 4                     # supers per quad (shared weights DMA/evict/out)
NCOL = HB * WS
NQ = TB // EV              # 14 quads

bf16 = ml_dtypes.bfloat16

_CACHE = {}


def build_nc(repeat=1):
    import contextlib
    import concourse.bass as bass  # noqa: F401
    import concourse.tile as tile
    from concourse import bacc, mybir

    dt = mybir.dt
    nc = bacc.Bacc("TRN2", target_bir_lowering=False, debug=False,
                   num_devices=N_CORES)
    q_ext = nc.dram_tensor("q", [NCOL, 128, NQ * EV * 128], dt.bfloat16,
                           kind="ExternalInput")
    k_ext = nc.dram_tensor("k", [NCOL, 128, KF], dt.bfloat16,
                           kind="ExternalInput")
    o_ext = nc.dram_tensor("o", [NCOL, 128, NQ * EV * BW], dt.bfloat16,
                           kind="ExternalOutput")

    with tile.TileContext(nc) as tc:
        with (
            tc.For_i(0, repeat, 1) if repeat > 1
            else contextlib.nullcontext(),
            tc.tile_pool(name="kpool", bufs=2) as kpool,
            tc.tile_pool(name="wpool", bufs=2) as wpool,
            tc.tile_pool(name="spool", bufs=2) as spool,
            tc.tile_pool(name="psum", bufs=2, space="PSUM") as pspool,
        ):
            for col in range(NCOL):
                kt = kpool.tile([128, KF], dt.bfloat16)
                nc.sync.dma_start(kt[:], k_ext[col])
                k4 = kt[:].rearrange("p (sh sw t) -> p sh sw t",
                                     sh=SH, sw=SW, t=ST)
                wt = wpool.tile([128, NQ * EV * 128], dt.bfloat16)
                nc.sync.dma_start(wt[:], q_ext[col])
                st = spool.tile([128, NQ * EV * BW], dt.bfloat16)
                for tq in range(NQ):
                    ps = pspool.tile([128, EV * 512], dt.float32)
                    for s in range(EV):
                        tb = tq * EV + s
                        rhs = k4[:, :, :, 2 * tb:2 * tb + 4]
                        nc.tensor.matmul(
                            ps[:, s * 512:s * 512 + NW],
                            wt[:, (tq * EV + s) * 128:(tq * EV + s + 1) * 128],
                            rhs)
                    ps4 = ps[:].rearrange("p (s n) -> p s n", s=EV, n=512)
                    st4 = st[:, tq * EV * BW:(tq + 1) * EV * BW].rearrange(
                        "p (s n) -> p s n", s=EV, n=BW)
                    eng = nc.vector.tensor_copy if tq % 2 == 0 \
                        else nc.scalar.copy
                    eng(st4[0:64], ps4[0:64, :, 0:BW])
                    eng(st4[64:128], ps4[64:128, :, 48:48 + BW])
                nc.sync.dma_start(o_ext[col], st[:])
    nc.compile()
    return nc


def prep_inputs(q, k):
    q = np.asarray(q, dtype=np.float32)
    k = np.asarray(k, dtype=np.float32)
    qs = (q * np.float32(1.0 / C)).astype(bf16)
    kpad = np.zeros((B, C, H + 2, W + 2, T + 2), dtype=bf16)
    kpad[:, :, 1:H + 1, 1:W + 1, 1:T + 1] = k.astype(bf16)
    in_maps = []
    for r in range(N_CORES):
        b = r // (N_CORES // B)
        h0 = (r % (N_CORES // B)) * HR
        # q blocks: qv[col, tb, g, c, v], v = i*8 + j*2 + l
        qb = qs[b, :, h0:h0 + HR]                 # (C, 20, 96, 112)
        s_c, s_h, s_w, s_t = qb.strides
        qv = np.lib.stride_tricks.as_strided(
            qb, shape=(HB, WS, TB, 4, C, AH, AW, AT),
            strides=(AH * s_h, 16 * s_w, AT * s_t, AW * s_w, s_c,
                     s_h, s_w, s_t))
        qv = qv.reshape(NCOL, TB, 4, C, 32)
        # materialized block-diagonal weights: Wm[col, tb, (g,c), (v,g')]
        Wm = np.zeros((NCOL, TB, 4, C, 32, 4), dtype=bf16)
        for g in range(4):
            Wm[:, :, g, :, :, g] = qv[:, :, g]
        Wm = Wm.reshape(NCOL, NQ * EV, 128, 128).transpose(0, 2, 1, 3)
        q_core = np.ascontiguousarray(Wm).reshape(NCOL, 128, NQ * EV * 128)
        # k slabs: [col, (g,c), (sh, sw, st)]
        kb = kpad[b, :, h0:h0 + HR + 2]           # (C, 22, 98, 114)
        s_c, s_h, s_w, s_t = kb.strides
        kv = np.lib.stride_tricks.as_strided(
            kb, shape=(HB, WS, 4, C, SH, SW, ST),
            strides=(AH * s_h, 16 * s_w, AW * s_w, s_c, s_h, s_w, s_t))
        k_core = np.ascontiguousarray(kv.reshape(NCOL, 128, KF))
        in_maps.append({"q": q_core, "k": k_core})
    return in_maps


def assemble_output(results):
    out = np.empty((B, 27, H, W, T), dtype=np.float32)
    # raw: [col, m, tq*EV*BW + s*BW + band-col], m = 32i+8j+4l+g,
    # band-col = (i%2)*24 + j*4 + l + off(tap), halves split by i>=2.
    core = np.empty((27, HB, WS, 2, 2, AH, AW, NQ, EV, AT), dtype=np.float32)
    s_m = NQ * EV * BW
    s_tq = EV * BW
    s_col = 128 * s_m
    for r in range(N_CORES):
        b = r // (N_CORES // B)
        h0 = (r % (N_CORES // B)) * HR
        raw = np.asarray(results[r]["o"])          # bf16
        flat = raw.reshape(-1)
        for dh in range(3):
            for dw in range(3):
                for dtt in range(3):
                    tap = dh * 9 + dw * 3 + dtt
                    off = dh * 24 + dw * 4 + dtt
                    for hi in range(2):            # i-half (i = 2*hi + ii)
                        view = np.lib.stride_tricks.as_strided(
                            flat[off + hi * 64 * s_m:],
                            shape=(NCOL, NQ, EV, 2, 4, 2, 4),
                            strides=tuple(2 * x for x in (
                                s_col, s_tq, BW,
                                32 * s_m + 24,     # ii
                                8 * s_m + 4,       # j
                                4 * s_m + 1,       # l
                                s_m)))             # g
                        # -> [hb, ws, hi, ii, g?, ...]
                        v = view.reshape(HB, WS, NQ, EV, 2, 4, 2, 4)
                        # dims: hb ws tq s ii j l g -> hb ws hi ii g j tq s l
                        core[tap, :, :, hi] = v.transpose(
                            0, 1, 4, 7, 5, 2, 3, 6)
        # core: tap, hb, ws, hi, ii, g, j, tq, s, l
        co = core.transpose(0, 1, 3, 4, 2, 5, 6, 7, 8, 9)
        out[b, :, h0:h0 + HR] = co.reshape(27, HR, W, T)
    return out


def kernel(q, k):
    from concourse.bass_utils import run_bass_kernel_spmd

    if "nc" not in _CACHE:
        _CACHE["nc"] = build_nc()
    nc = _CACHE["nc"]
    in_maps = prep_inputs(q, k)
    res = run_bass_kernel_spmd(nc, in_maps, core_ids=list(range(N_CORES)))
    return assemble_output(res.results)
